# revision 42
# baseline (speedup 1.0000x reference)
"""Adder2D (L1-distance "convolution") Trainium2 Bass kernel, 8 NeuronCores.

out[n, f, ho, wo] = -sum_d |W[f, d] - X_col[d, (n, ho, wo)]|
with d = (c, dy, dx), C=128, 3x3 kernel, stride 1, pad 1.

Sharding: output-channel tensor parallel. Core i computes filters
[16*i, 16*(i+1)); every core sees the full x. No collectives; the host
concatenates the 8 per-core outputs along the filter axis.

Per-core algorithm (relu identity, exact):
  |x-w| = 2*relu(x-w) - (x-w)
  out[f, l] = -2*sum_d relu(x - w[f,d]) + S_X[l] - S_W[f]
  S_X[l] = sum_d x[d, l],  S_W[f] = sum_d w[f, d]

  - x (f32) is DMA'd contiguously, then zero-padded into
    [128c, 8n*18*18] on-chip.
  - 9 shifted copies materialize im2col patches as bf16 [128c, 2048l];
    the "unfold" is just an access pattern (center patch first: it has
    no dependency on the padding, so the PE pipeline starts early).
  - relu tiles: DVE tensor_scalar(op0=subtract, op1=max, 0.0) with a
    per-partition f32 W scalar (4x bf16 path), ~75% of tiles; ACT
    activation(Relu, bias=-w) for the rest.
  - TensorEngine reduces over partitions with accumulating matmuls into
    one [16, 2048] PSUM tile; stationary = [128,16] column of -2 at
    column f. A custom pass drops LDWEIGHTS whose stationary is
    unchanged (else walrus reloads it for every matmul: +38% PE time).
  - S_X: tree-add of the 9 patches (DVE/GpSimd) + a ones-column matmul;
    S_W: ones-column matmul over W + a 9-fold strided add. Both are
    broadcast back into the PSUM accumulator with K=1 matmuls, so the
    corrections cost ~13 matmuls instead of 72.
"""

import numpy as np

N, C, H, W_ = 8, 128, 16, 16
F, KH, KW = 128, 3, 3
NCORES = 8
FL = F // NCORES          # 16 filters per core
HP, WP = H + 2, W_ + 2    # padded 18x18
L = N * H * W_            # 2048 output columns
DCH = KH * KW             # 9 shift chunks of 128 channels
NT = 512                  # matmul moving free dim (one PSUM bank)
WARMUP_MM = 6             # PE warmup matmuls during the DMA/setup phase
# Per-filter tile plan:
#  - DVE bf16: singles j4, j6; pair (0,1) merged via TT-add always,
#    pair (2,3) merged on even f (else fed as singles)
#  - ACT: js (5,7) as one fp8 pair -> DoubleRow matmuls; j8 bf16 single
MERGE_ALWAYS = (0, 1)
MERGE_EVEN_F = (2, 3)
FP8_PAIR = (5, 7)
# patch creation order == f0's consumption order (lazy interleave)
J_ORDER = [4, 6, 0, 1, 2, 3, 5, 7, 8]

_CACHE = {}


def _dedup_ldweights(nc):
    """Drop InstLdweights whose stationary operand is identical to the
    previous weight load on the PE stream (the array keeps its weights
    between matmuls; per-matmul reloads of an unchanged stationary are
    pure overhead). Runs after Tile scheduling, before bacc.compile,
    when the ldweights carry no semaphore sync."""
    from concourse import mybir
    removed = 0
    for fn in nc.m.functions:
        for blk in fn.blocks:
            last_key = None
            keep = []
            for inst in blk.instructions:
                if isinstance(inst, mybir.InstLdweights):
                    si = inst.sync_info
                    clean = si is None or (not si.on_wait and not si.on_update)
                    key = "|".join(str(s) for s in (
                        inst.ins[0], inst.perf_mode, inst.is_transpose,
                        inst.tile_position, inst.tile_size))
                    if clean and key == last_key:
                        removed += 1
                        continue
                    last_key = key
                keep.append(inst)
            blk.instructions[:] = keep
    return removed


def _build_nc():
    from concourse import bacc, mybir
    import concourse.tile as tile

    f32 = mybir.dt.float32
    bf16 = mybir.dt.bfloat16
    fp8 = mybir.dt.float8e4
    Alu = mybir.AluOpType
    Act = mybir.ActivationFunctionType

    nc = bacc.Bacc("TRN2", target_bir_lowering=False, debug=False,
                   num_devices=NCORES)
    x_d = nc.dram_tensor("x", [N, C, H, W_], f32, kind="ExternalInput")
    w_d = nc.dram_tensor("w", [FL, C, KH, KW], f32, kind="ExternalInput")
    out_d = nc.dram_tensor("out", [N, FL, H, W_], f32, kind="ExternalOutput")

    with tile.TileContext(nc) as tc:
        with tc.tile_pool(name="setup", bufs=1) as sp, \
             tc.tile_pool(name="diff", bufs=8) as dp, \
             tc.tile_pool(name="psum", bufs=1, space="PSUM") as pp:

            # ---- PE warmup first: constants on DVE, then matmuls that
            #      keep HAM at 2.4 GHz while the DMAs/setup run ----
            ones_st = sp.tile([128, FL], bf16)
            nc.vector.memset(ones_st[:], 1.0)
            neg_ones = sp.tile([128, NT], bf16)
            nc.vector.memset(neg_ones[:], -1.0)
            # preload the ACT spline tables before the first real Relu
            actwarm = sp.tile([1, 16], f32)
            nc.scalar.activation(actwarm[:], ones_st[0:1, 0:16], Act.Relu)

            # ---- W first (tiny), then x: contiguous DMAs ----
            w_raw = sp.tile([FL, C * DCH], f32)
            nc.sync.dma_start(w_raw[:], w_d.ap().rearrange(
                "f c kh kw -> f (c kh kw)"))
            x_flat = sp.tile([128, L], f32)
            xsrc = x_d.ap().rearrange("n c h w -> c n (h w)")
            x_flat3 = x_flat[:].rearrange("p (n hw) -> p n hw", n=N)
            for n in range(N):      # alternate the two HW DGE queues
                eng = nc.sync if n % 2 == 0 else nc.scalar
                eng.dma_start(x_flat3[:, n, :], xsrc[:, n, :])

            # ---- W transposed on the (idle) PE: 9 shifts of [16, 128]
            #      -> [128c, (j f)] in PSUM; then warmup matmuls keep
            #      HAM at 2.4 GHz while the rest of setup runs ----
            from concourse.masks import make_identity
            ident = sp.tile([FL, FL], f32)
            make_identity(nc, ident[:])
            wtp = pp.tile([128, DCH * FL], f32, tag="wt")
            w_raw3 = w_raw[:].rearrange("p (c j) -> p c j", j=DCH)
            for j in range(DCH):
                nc.tensor.matmul(
                    wtp[:, FL * j:FL * (j + 1)], w_raw3[:, :, j], ident[:],
                    is_transpose=True, start=True, stop=True)
            warm = pp.tile([FL, NT], f32, tag="aux")
            for i in range(WARMUP_MM):
                nc.tensor.matmul(warm[:], ones_st[:], neg_ones[:],
                                 start=(i == 0), stop=(i == WARMUP_MM - 1))

            w32 = sp.tile([128, DCH * FL], f32)
            nc.vector.tensor_copy(w32[:], wtp[:])
            w32n = sp.tile([128, DCH * FL], f32)
            nc.vector.tensor_scalar(w32n[:], w32[:], -1.0, None, op0=Alu.mult)
            w32_3 = w32[:].rearrange("p (j f) -> p j f", j=DCH)
            w32n_3 = w32n[:].rearrange("p (j f) -> p j f", j=DCH)

            # ---- stationary / constant tiles ----
            ind = sp.tile([128, FL * FL], bf16)   # -2 at column f
            nc.gpsimd.memset(ind[:], 0.0)
            ind3 = ind[:].rearrange("p (f m) -> p f m", f=FL)
            for f in range(FL):
                nc.gpsimd.memset(ind3[:, f, f:f + 1], -2.0)
            # fp8 DoubleRow stationary: -2 at column f for both virtual
            # K-rows (built in bf16, cast to fp8)
            ind8b = sp.tile([128, FL * 2 * FL], bf16)
            nc.gpsimd.memset(ind8b[:], 0.0)
            ind8b4 = ind8b[:].rearrange("p (f r m) -> p f r m", f=FL, r=2)
            for f in range(FL):
                for r in range(2):
                    nc.gpsimd.memset(ind8b4[:, f, r, f:f + 1], -2.0)
            ind8 = sp.tile([128, FL * 2 * FL], fp8)
            nc.gpsimd.tensor_copy(ind8[:], ind8b[:])
            ind8_4 = ind8[:].rearrange("p (f r m) -> p f r m", f=FL, r=2)
            # ---- padded x ----
            x_pad = sp.tile([128, N * HP * WP], f32)
            nc.gpsimd.memset(x_pad[:], 0.0)
            x_pad4 = x_pad[:].rearrange("p (n h w) -> p n h w", n=N, h=HP, w=WP)
            nc.vector.tensor_copy(
                x_pad4[:, :, 1:1 + H, 1:1 + W_],
                x_flat[:].rearrange("p (n h w) -> p n h w", n=N, h=H, w=W_))

            # ---- the 9 shifted patch tiles (bf16), created lazily in
            #      f0's consumption order so PE is fed immediately ----
            patches = [None] * DCH

            def ensure_patch(j):
                if patches[j] is not None:
                    return
                k = J_ORDER.index(j)
                dy, dx = divmod(j, KW)
                pj = sp.tile([128, L], bf16, tag=f"patch{j}")
                if j == 4:
                    nc.vector.tensor_copy(pj[:], x_flat[:])
                else:
                    pj4 = pj[:].rearrange(
                        "p (n h w) -> p n h w", n=N, h=H, w=W_)
                    src = x_pad4[:, :, dy:dy + H, dx:dx + W_]
                    if k in (2, 4):
                        nc.scalar.copy(pj4, src)
                    else:
                        nc.vector.tensor_copy(pj4, src)
                patches[j] = pj

            psum = pp.tile([FL, L], f32)
            nchunks = L // NT

            # ---- main loop: relu tiles -> accumulating matmuls.
            #      DVE makes bf16 tiles (some pre-added pairs); ACT
            #      makes an fp8 pair per f fed via DoubleRow matmuls
            #      plus one bf16 single. ----
            first = [True] * nchunks

            def dve_tile(f, j):
                dt_ = dp.tile([128, L], bf16, tag="diff")
                nc.vector.tensor_scalar(
                    dt_[:], patches[j][:], w32_3[:, j, f:f + 1], 0.0,
                    op0=Alu.subtract, op1=Alu.max)
                return dt_

            def act_tile(f, j):
                dt_ = dp.tile([128, L], bf16, tag="diff")
                nc.scalar.activation(
                    dt_[:], patches[j][:], Act.Relu,
                    bias=w32n_3[:, j, f:f + 1], scale=1.0)
                return dt_

            def feed_pe(dt_, lhsT):
                for ncnk in range(nchunks):
                    cs = slice(ncnk * NT, (ncnk + 1) * NT)
                    nc.tensor.matmul(
                        psum[:, cs], lhsT, dt_[:, cs],
                        start=first[ncnk], stop=False)
                    first[ncnk] = False

            for f in range(FL):
                lhsT = ind3[:, f, :]
                for j in (4, 6):
                    ensure_patch(j)
                    feed_pe(dve_tile(f, j), lhsT)
                pairs = [MERGE_ALWAYS]
                extras = []
                if f % 2 == 0:
                    pairs.append(MERGE_EVEN_F)
                else:
                    extras = list(MERGE_EVEN_F)
                for ja, jb in pairs:
                    ensure_patch(ja)
                    ensure_patch(jb)
                    da = dve_tile(f, ja)
                    db = dve_tile(f, jb)
                    nc.vector.tensor_tensor(da[:], da[:], db[:], op=Alu.add)
                    feed_pe(da, lhsT)
                for j in extras:
                    ensure_patch(j)
                    feed_pe(dve_tile(f, j), lhsT)
                # ACT bf16 single (same stationary, no LDW switch)
                ensure_patch(8)
                feed_pe(act_tile(f, 8), lhsT)
                # ACT fp8 pair -> DoubleRow
                ja, jb = FP8_PAIR
                ensure_patch(ja)
                ensure_patch(jb)
                fpair = dp.tile([128, 2 * L], fp8, tag="fpair")
                fp3 = fpair[:].rearrange("p (r l) -> p r l", r=2)
                nc.scalar.activation(
                    fp3[:, 0, :], patches[ja][:], Act.Relu,
                    bias=w32n_3[:, ja, f:f + 1], scale=1.0)
                nc.scalar.activation(
                    fp3[:, 1, :], patches[jb][:], Act.Relu,
                    bias=w32n_3[:, jb, f:f + 1], scale=1.0)
                for ncnk in range(nchunks):
                    cs = slice(ncnk * NT, (ncnk + 1) * NT)
                    nc.tensor.matmul(
                        psum[:, cs], ind8_4[:, f, :, :], fp3[:, :, cs],
                        perf_mode=mybir.MatmulPerfMode.DoubleRow,
                        start=first[ncnk], stop=False)
                    first[ncnk] = False

            # ---- corrections (all emitted after the main loop: engine
            #      queues are FIFO, so anything waiting here cannot
            #      block the main pipeline) ----
            # S_X: += sum_d(chunk j) x[d, l] for every row, one
            # stationary (ones) for all 36 matmuls
            for j in range(DCH):
                for ncnk in range(nchunks):
                    cs = slice(ncnk * NT, (ncnk + 1) * NT)
                    nc.tensor.matmul(
                        psum[:, cs], ones_st[:], patches[j][:, cs],
                        start=False, stop=False)

            # S_W: reduce W over partitions, fold the 9 taps
            wb = sp.tile([128, DCH * FL], bf16)
            nc.vector.tensor_copy(wb[:], w32[:])
            swp = pp.tile([1, FL * DCH], f32, tag="aux")
            nc.tensor.matmul(swp[:], ones_st[:, 0:1], wb[:],
                             start=True, stop=True)
            swf = sp.tile([1, FL * DCH], f32)
            nc.scalar.copy(swf[:], swp[:])
            swf3 = swf[:].rearrange("p (j f) -> p j f", j=DCH)
            for k in range(1, DCH):
                nc.vector.tensor_tensor(
                    swf3[:, 0, :], swf3[:, 0, :], swf3[:, k, :], op=Alu.add)
            swb = sp.tile([1, FL], bf16)
            nc.vector.tensor_copy(swb[:], swf3[:, 0, :])

            # broadcast -S_W into psum with K=1 matmuls, then stream
            # each finished chunk straight out
            osb = sp.tile([FL, L], f32)
            odst = out_d.ap().rearrange("n f h w -> f n (h w)")
            osb3 = osb[:].rearrange("f (n hw) -> f n hw", n=N)
            for ncnk in range(nchunks):
                cs = slice(ncnk * NT, (ncnk + 1) * NT)
                nc.tensor.matmul(                      # += -S_W[f] every col
                    psum[:, cs], swb[:], neg_ones[0:1, :],
                    start=False, stop=True)
                nc.scalar.copy(osb[:, cs], psum[:, cs])
                ns_ = slice(2 * ncnk, 2 * ncnk + 2)
                nc.sync.dma_start(odst[:, ns_, :], osb3[:, ns_, :])

    _dedup_ldweights(nc)
    nc.compile()
    return nc


def kernel(x, W):
    x = np.ascontiguousarray(np.asarray(x, dtype=np.float32))
    W = np.ascontiguousarray(np.asarray(W, dtype=np.float32))
    assert x.shape == (N, C, H, W_) and W.shape == (F, C, KH, KW)

    if "nc" not in _CACHE:
        _CACHE["nc"] = _build_nc()
    nc = _CACHE["nc"]

    from concourse.bass_utils import run_bass_kernel_spmd

    in_maps = [
        {"x": x, "w": np.ascontiguousarray(W[FL * i:FL * (i + 1)])}
        for i in range(NCORES)
    ]
    trace = bool(_CACHE.get("trace", False))
    res = run_bass_kernel_spmd(nc, in_maps, core_ids=list(range(NCORES)),
                               trace=trace)
    _CACHE["exec_time_ns"] = res.exec_time_ns
    out = np.concatenate([r["out"] for r in res.results], axis=1)
    return out.astype(np.float32)


# revision 45
# speedup vs baseline: 1.2015x; 1.2015x over previous
"""Adder2D (L1-distance "convolution") Trainium2 Bass kernel, 8 NeuronCores.

out[n, f, ho, wo] = -sum_d |W[f, d] - X_col[d, (n, ho, wo)]|
with d = (c, dy, dx), C=128, 3x3 kernel, stride 1, pad 1.

Sharding: output-channel tensor parallel. Core i computes filters
[16*i, 16*(i+1)); every core sees the full x. No collectives; the host
concatenates the 8 per-core outputs along the filter axis.

Per-core algorithm (relu identity, exact):
  |x-w| = 2*relu(x-w) - (x-w)
  out[f, l] = -2*sum_d relu(x - w[f,d]) + S_X[l] - S_W[f]
  S_X[l] = sum_d x[d, l],  S_W[f] = sum_d w[f, d]

  - x (f32) is DMA'd contiguously, then zero-padded into
    [128c, 8n*18*18] on-chip.
  - 9 shifted copies materialize im2col patches as bf16 [128c, 2048l];
    the "unfold" is just an access pattern (center patch first: it has
    no dependency on the padding, so the PE pipeline starts early).
  - relu tiles: DVE tensor_scalar(op0=subtract, op1=max, 0.0) with a
    per-partition f32 W scalar (4x bf16 path), ~75% of tiles; ACT
    activation(Relu, bias=-w) for the rest.
  - TensorEngine reduces over partitions with accumulating matmuls into
    one [16, 2048] PSUM tile; stationary = [128,16] column of -2 at
    column f. A custom pass drops LDWEIGHTS whose stationary is
    unchanged (else walrus reloads it for every matmul: +38% PE time).
  - S_X: tree-add of the 9 patches (DVE/GpSimd) + a ones-column matmul;
    S_W: ones-column matmul over W + a 9-fold strided add. Both are
    broadcast back into the PSUM accumulator with K=1 matmuls, so the
    corrections cost ~13 matmuls instead of 72.
"""

import numpy as np

N, C, H, W_ = 8, 128, 16, 16
F, KH, KW = 128, 3, 3
NCORES = 8
FL = F // NCORES          # 16 filters per core
HP, WP = H + 2, W_ + 2    # padded 18x18
L = N * H * W_            # 2048 output columns
DCH = KH * KW             # 9 shift chunks of 128 channels
NT = 512                  # matmul moving free dim (one PSUM bank)
WARMUP_MM = 3             # PE warmup matmuls during the DMA/setup phase
# Per-filter tile plan:
#  - DVE bf16: singles j4, j6; pair (0,1) merged via TT-add always,
#    pair (2,3) merged on even f (else fed as singles)
#  - ACT: js (5,7) as one fp8 pair -> DoubleRow matmuls; j8 bf16 single
MERGE_ALWAYS = (0, 1)
MERGE_EVEN_F = (2, 3)
FP8_PAIR = (5, 7)
# patch creation order == f0's consumption order (lazy interleave)
J_ORDER = [4, 6, 0, 1, 2, 3, 5, 7, 8]

_CACHE = {}


def _dedup_ldweights(nc):
    """Drop InstLdweights whose stationary operand is identical to the
    previous weight load on the PE stream (the array keeps its weights
    between matmuls; per-matmul reloads of an unchanged stationary are
    pure overhead). Runs after Tile scheduling, before bacc.compile,
    when the ldweights carry no semaphore sync."""
    from concourse import mybir
    removed = 0
    for fn in nc.m.functions:
        for blk in fn.blocks:
            last_key = None
            keep = []
            for inst in blk.instructions:
                if isinstance(inst, mybir.InstLdweights):
                    si = inst.sync_info
                    clean = si is None or (not si.on_wait and not si.on_update)
                    key = "|".join(str(s) for s in (
                        inst.ins[0], inst.perf_mode, inst.is_transpose,
                        inst.tile_position, inst.tile_size))
                    if clean and key == last_key:
                        removed += 1
                        continue
                    last_key = key
                keep.append(inst)
            blk.instructions[:] = keep
    return removed


def _build_nc():
    from concourse import bacc, mybir
    import concourse.tile as tile

    f32 = mybir.dt.float32
    bf16 = mybir.dt.bfloat16
    fp8 = mybir.dt.float8e4
    Alu = mybir.AluOpType
    Act = mybir.ActivationFunctionType

    nc = bacc.Bacc("TRN2", target_bir_lowering=False, debug=False,
                   num_devices=NCORES)
    x_d = nc.dram_tensor("x", [N, C, H, W_], f32, kind="ExternalInput")
    w_d = nc.dram_tensor("w", [FL, C, KH, KW], f32, kind="ExternalInput")
    out_d = nc.dram_tensor("out", [N, FL, H, W_], f32, kind="ExternalOutput")

    with tile.TileContext(nc) as tc:
        with tc.tile_pool(name="setup", bufs=1) as sp, \
             tc.tile_pool(name="diff", bufs=8) as dp, \
             tc.tile_pool(name="psum", bufs=1, space="PSUM") as pp:

            # ---- PE warmup first: constants on DVE, then matmuls that
            #      keep HAM at 2.4 GHz while the DMAs/setup run ----
            ones_st = sp.tile([128, FL], bf16)
            nc.vector.memset(ones_st[:], 1.0)
            neg_ones = sp.tile([128, NT], bf16)
            nc.vector.memset(neg_ones[:], -1.0)
            # preload the ACT spline tables before the first real Relu
            actwarm = sp.tile([1, 16], f32)
            nc.scalar.activation(actwarm[:], ones_st[0:1, 0:16], Act.Relu)

            # ---- W first (tiny), then x: contiguous DMAs ----
            w_raw = sp.tile([FL, C * DCH], f32)
            nc.sync.dma_start(w_raw[:], w_d.ap().rearrange(
                "f c kh kw -> f (c kh kw)"))
            x_flat = sp.tile([128, L], f32)
            xsrc = x_d.ap().rearrange("n c h w -> c n (h w)")
            x_flat3 = x_flat[:].rearrange("p (n hw) -> p n hw", n=N)
            for n in range(N):      # alternate the two HW DGE queues
                eng = nc.sync if n % 2 == 0 else nc.scalar
                eng.dma_start(x_flat3[:, n, :], xsrc[:, n, :])

            # ---- W transposed on the (idle) PE: 9 shifts of [16, 128]
            #      -> [128c, (j f)] in PSUM; then warmup matmuls keep
            #      HAM at 2.4 GHz while the rest of setup runs ----
            from concourse.masks import make_identity
            ident = sp.tile([FL, FL], f32)
            make_identity(nc, ident[:])
            wtp = pp.tile([128, DCH * FL], f32, tag="wt")
            w_raw3 = w_raw[:].rearrange("p (c j) -> p c j", j=DCH)
            for j in range(DCH):
                nc.tensor.matmul(
                    wtp[:, FL * j:FL * (j + 1)], w_raw3[:, :, j], ident[:],
                    is_transpose=True, start=True, stop=True)
            warm = pp.tile([FL, NT], f32, tag="aux")
            for i in range(WARMUP_MM):
                nc.tensor.matmul(warm[:], ones_st[:], neg_ones[:],
                                 start=(i == 0), stop=(i == WARMUP_MM - 1))

            # ---- padded x (memset first on GpSimd's queue) ----
            x_pad = sp.tile([128, N * HP * WP], f32)
            nc.gpsimd.memset(x_pad[:], 0.0)
            x_pad4 = x_pad[:].rearrange("p (n h w) -> p n h w", n=N, h=HP, w=WP)
            nc.vector.tensor_copy(
                x_pad4[:, :, 1:1 + H, 1:1 + W_],
                x_flat[:].rearrange("p (n h w) -> p n h w", n=N, h=H, w=W_))

            # ---- the 9 shifted patch tiles (bf16), created lazily in
            #      f0's consumption order so PE is fed immediately ----
            patches = [None] * DCH

            def ensure_patch(j):
                if patches[j] is not None:
                    return
                k = J_ORDER.index(j)
                dy, dx = divmod(j, KW)
                pj = sp.tile([128, L], bf16, tag=f"patch{j}")
                if j == 4:
                    nc.vector.tensor_copy(pj[:], x_flat[:])
                else:
                    pj4 = pj[:].rearrange(
                        "p (n h w) -> p n h w", n=N, h=H, w=W_)
                    src = x_pad4[:, :, dy:dy + H, dx:dx + W_]
                    if k in (2, 4):
                        nc.scalar.copy(pj4, src)
                    else:
                        nc.vector.tensor_copy(pj4, src)
                patches[j] = pj

            # first two patches ahead of the W chain on DVE's queue, so
            # they land while the w32 copy waits for the PE transposes
            ensure_patch(4)
            ensure_patch(6)

            w32 = sp.tile([128, DCH * FL], f32)
            nc.vector.tensor_copy(w32[:], wtp[:])
            w32n = sp.tile([128, DCH * FL], f32)
            nc.vector.tensor_scalar(w32n[:], w32[:], -1.0, None, op0=Alu.mult)
            w32_3 = w32[:].rearrange("p (j f) -> p j f", j=DCH)
            w32n_3 = w32n[:].rearrange("p (j f) -> p j f", j=DCH)

            # ---- S_W pipeline early (GpSimd folds, PE fills its own
            #      setup gap) so the tail corrections never stall ----
            wb = sp.tile([128, DCH * FL], bf16)
            nc.gpsimd.tensor_copy(wb[:], w32[:])
            swp = pp.tile([1, FL * DCH], f32, tag="aux")
            nc.tensor.matmul(swp[:], ones_st[:, 0:1], wb[:],
                             start=True, stop=True)
            swf = sp.tile([1, FL * DCH], f32)
            nc.scalar.copy(swf[:], swp[:])
            swf3 = swf[:].rearrange("p (j f) -> p j f", j=DCH)
            for k in range(1, DCH):
                nc.gpsimd.tensor_tensor(
                    swf3[:, 0, :], swf3[:, 0, :], swf3[:, k, :], op=Alu.add)
            swb = sp.tile([1, FL], bf16)
            nc.gpsimd.tensor_copy(swb[:], swf3[:, 0, :])

            # ---- stationary / constant tiles ----
            ind = sp.tile([128, FL * FL], bf16)   # -2 at column f
            nc.gpsimd.memset(ind[:], 0.0)
            ind3 = ind[:].rearrange("p (f m) -> p f m", f=FL)
            for f in range(FL):
                nc.gpsimd.memset(ind3[:, f, f:f + 1], -2.0)
            # fp8 DoubleRow stationary: -2 at column f for both virtual
            # K-rows (built in bf16, cast to fp8)
            ind8b = sp.tile([128, FL * 2 * FL], bf16)
            nc.gpsimd.memset(ind8b[:], 0.0)
            ind8b4 = ind8b[:].rearrange("p (f r m) -> p f r m", f=FL, r=2)
            for f in range(FL):
                for r in range(2):
                    nc.gpsimd.memset(ind8b4[:, f, r, f:f + 1], -2.0)
            ind8 = sp.tile([128, FL * 2 * FL], fp8)
            nc.gpsimd.tensor_copy(ind8[:], ind8b[:])
            ind8_4 = ind8[:].rearrange("p (f r m) -> p f r m", f=FL, r=2)

            psum = pp.tile([FL, L], f32)
            nchunks = L // NT

            # ---- main loop: relu tiles -> accumulating matmuls.
            #      DVE makes bf16 tiles (some pre-added pairs); ACT
            #      makes an fp8 pair per f fed via DoubleRow matmuls
            #      plus one bf16 single. ----
            first = [True] * nchunks

            def dve_tile(f, j):
                dt_ = dp.tile([128, L], bf16, tag="diff")
                nc.vector.tensor_scalar(
                    dt_[:], patches[j][:], w32_3[:, j, f:f + 1], 0.0,
                    op0=Alu.subtract, op1=Alu.max)
                return dt_

            def act_tile(f, j):
                dt_ = dp.tile([128, L], bf16, tag="diff")
                nc.scalar.activation(
                    dt_[:], patches[j][:], Act.Relu,
                    bias=w32n_3[:, j, f:f + 1], scale=1.0)
                return dt_

            def feed_pe(dt_, lhsT):
                for ncnk in range(nchunks):
                    cs = slice(ncnk * NT, (ncnk + 1) * NT)
                    nc.tensor.matmul(
                        psum[:, cs], lhsT, dt_[:, cs],
                        start=first[ncnk], stop=False)
                    first[ncnk] = False

            for f in range(FL):
                lhsT = ind3[:, f, :]
                for j in (4, 6):
                    ensure_patch(j)
                    feed_pe(dve_tile(f, j), lhsT)
                pairs = [MERGE_ALWAYS]
                extras = []
                if f % 2 == 0:
                    pairs.append(MERGE_EVEN_F)
                else:
                    extras = list(MERGE_EVEN_F)
                for ja, jb in pairs:
                    ensure_patch(ja)
                    ensure_patch(jb)
                    da = dve_tile(f, ja)
                    db = dve_tile(f, jb)
                    nc.vector.tensor_tensor(da[:], da[:], db[:], op=Alu.add)
                    feed_pe(da, lhsT)
                for j in extras:
                    ensure_patch(j)
                    feed_pe(dve_tile(f, j), lhsT)
                # ACT bf16 single (same stationary, no LDW switch)
                ensure_patch(8)
                feed_pe(act_tile(f, 8), lhsT)
                # ACT fp8 pair -> DoubleRow
                ja, jb = FP8_PAIR
                ensure_patch(ja)
                ensure_patch(jb)
                fpair = dp.tile([128, 2 * L], fp8, tag="fpair")
                fp3 = fpair[:].rearrange("p (r l) -> p r l", r=2)
                nc.scalar.activation(
                    fp3[:, 0, :], patches[ja][:], Act.Relu,
                    bias=w32n_3[:, ja, f:f + 1], scale=1.0)
                nc.scalar.activation(
                    fp3[:, 1, :], patches[jb][:], Act.Relu,
                    bias=w32n_3[:, jb, f:f + 1], scale=1.0)
                for ncnk in range(nchunks):
                    cs = slice(ncnk * NT, (ncnk + 1) * NT)
                    nc.tensor.matmul(
                        psum[:, cs], ind8_4[:, f, :, :], fp3[:, :, cs],
                        perf_mode=mybir.MatmulPerfMode.DoubleRow,
                        start=first[ncnk], stop=False)
                    first[ncnk] = False

            # ---- corrections (all emitted after the main loop: engine
            #      queues are FIFO, so anything waiting here cannot
            #      block the main pipeline) ----
            # S_X: += sum_d(chunk j) x[d, l] for every row, one
            # stationary (ones) for all 36 matmuls
            for j in range(DCH):
                for ncnk in range(nchunks):
                    cs = slice(ncnk * NT, (ncnk + 1) * NT)
                    nc.tensor.matmul(
                        psum[:, cs], ones_st[:], patches[j][:, cs],
                        start=False, stop=False)


            # broadcast -S_W into psum with K=1 matmuls, then stream
            # each finished chunk straight out
            osb = sp.tile([FL, L], f32)
            odst = out_d.ap().rearrange("n f h w -> f n (h w)")
            osb3 = osb[:].rearrange("f (n hw) -> f n hw", n=N)
            for ncnk in range(nchunks):
                cs = slice(ncnk * NT, (ncnk + 1) * NT)
                nc.tensor.matmul(                      # += -S_W[f] every col
                    psum[:, cs], swb[:], neg_ones[0:1, :],
                    start=False, stop=True)
                nc.scalar.copy(osb[:, cs], psum[:, cs])
                ns_ = slice(2 * ncnk, 2 * ncnk + 2)
                nc.sync.dma_start(odst[:, ns_, :], osb3[:, ns_, :])

    _dedup_ldweights(nc)
    nc.compile()
    return nc


def kernel(x, W):
    x = np.ascontiguousarray(np.asarray(x, dtype=np.float32))
    W = np.ascontiguousarray(np.asarray(W, dtype=np.float32))
    assert x.shape == (N, C, H, W_) and W.shape == (F, C, KH, KW)

    if "nc" not in _CACHE:
        _CACHE["nc"] = _build_nc()
    nc = _CACHE["nc"]

    from concourse.bass_utils import run_bass_kernel_spmd

    in_maps = [
        {"x": x, "w": np.ascontiguousarray(W[FL * i:FL * (i + 1)])}
        for i in range(NCORES)
    ]
    trace = bool(_CACHE.get("trace", False))
    res = run_bass_kernel_spmd(nc, in_maps, core_ids=list(range(NCORES)),
                               trace=trace)
    _CACHE["exec_time_ns"] = res.exec_time_ns
    out = np.concatenate([r["out"] for r in res.results], axis=1)
    return out.astype(np.float32)
